# revision 1
# baseline (speedup 1.0000x reference)
"""GAT 4-layer model on 8 Trainium2 NeuronCores (Bass/Tile).

Strategy (dst-sharded node-parallel):
  - 20000 nodes -> 8 cores x 2500 nodes (padded to 2560 rows = 20 windows of 128).
  - Edges grouped by dst node; each core owns all in-edges of its nodes.
  - Nodes within a core are reordered by in-degree so each 128-node window has
    a small uniform max degree S[w]; per-node edge slots padded to S[w] with a
    dedicated all-zero row (el = -1e30 -> exp underflows to 0 contribution).
  - Key algebraic trick: sum_e alpha_e * (x[src_e] @ W) == (sum_e alpha_e * x[src_e]) @ W,
    so we aggregate RAW input rows (din wide) and apply W once per dst window.
  - dma_gather pulls x[src] rows so that slot s of dst-partition v holds that
    node's s-th in-edge row => segment softmax = per-partition free-dim reduce.
  - After each layer, an AllGather replicates the new node features (fp16 rows,
    fp32 el appended) to every core for the next layer's gathers.
"""

import os
import sys

sys.path.insert(0, "/opt/trn_rl_repo")

import numpy as np

N = 20000
E = 320000
C = 64
DIN = [64, 128, 256, 512]     # per layer input dim
DOUT = [128, 256, 512, 1024]  # per layer output dim
NCORES = 8
NPC = 2500        # real nodes per core
RPC = 2560        # rows per core (20 windows x 128)
NW = 20
PAD_LO, PAD_HI = 1476, 1536   # pad positions inside each core's 2560 rows
HALF = RPC // 2               # split-AllGather half size (rows per core per AG)
# zero row = core 0, pos 1476 (>= HALF) in half-major xfull layout
ZROW = NCORES * HALF + (1476 - HALF)
NEG_SLOPE = 0.2
NEG_BIG = -1.0e30

# fp16 row layout per layer: [x (din) fp16 | el fp32 (2 slots) | pad] to 128-elem mult
COLS16 = [128, 256, 384, 640]
EL32 = [d // 2 for d in DIN]  # fp32-view column index of el


def _prep_graph(src, dst):
    """Host preprocessing: node permutation, window degrees, gather indices."""
    deg = np.bincount(dst, minlength=N)
    order = np.argsort(dst, kind="stable")
    src_s = src[order]
    ptr = np.zeros(N + 1, np.int64)
    ptr[1:] = np.cumsum(deg)

    pos2node = np.full((NCORES, RPC), -1, np.int64)
    for k in range(NCORES):
        nodes = np.arange(k * NPC, (k + 1) * NPC)
        if k == 0:
            pool_nodes = np.arange(0, 1024)           # pooling nodes, original order
            rest = np.arange(1024, 2500)
            rest = rest[np.argsort(deg[rest], kind="stable")]
            pos2node[0, :PAD_LO] = rest
            pos2node[0, PAD_HI:] = pool_nodes
        else:
            sn = nodes[np.argsort(deg[nodes], kind="stable")]
            pos2node[k, :PAD_LO] = sn[:PAD_LO]
            pos2node[k, PAD_HI:] = sn[PAD_LO:]

    # xfull row layout is split-AllGather half-major:
    #   pos <  1280: row = k*1280 + pos
    #   pos >= 1280: row = 10240 + k*1280 + (pos - 1280)
    node2row = np.zeros(N, np.int64)
    for k in range(NCORES):
        m = pos2node[k] >= 0
        pos = np.nonzero(m)[0]
        row = np.where(pos < HALF, k * HALF + pos,
                       NCORES * HALF + k * HALF + (pos - HALF))
        node2row[pos2node[k][m]] = row

    S = np.zeros(NW, np.int64)
    for w in range(NW):
        mx = 2
        nd = pos2node[:, w * 128:(w + 1) * 128].ravel()
        nd = nd[nd >= 0]
        if nd.size:
            mx = max(mx, int(deg[nd].max()))
        S[w] = mx + (mx % 2)

    NIDX = int(128 * S.sum())
    gidx = np.full((NCORES, NIDX), ZROW, np.int32)
    base = 0
    for w in range(NW):
        sw = int(S[w])
        for k in range(NCORES):
            for p in range(128):
                node = pos2node[k, w * 128 + p]
                if node < 0:
                    continue
                d = int(deg[node])
                if d == 0:
                    continue
                rows = node2row[src_s[ptr[node]:ptr[node + 1]]]
                gidx[k, base + np.arange(d) * 128 + p] = rows
        base += 128 * sw
    assert gidx.max() < 32768

    # wrap to dma_gather layout: idx j -> [j%16, j//16], replicated to 128 partitions
    w16 = gidx.reshape(NCORES, NIDX // 16, 16).transpose(0, 2, 1)
    gidx16 = np.tile(w16, (1, 8, 1)).astype(np.int16)
    return pos2node, node2row, S, gidx16


def _zrow(layer):
    z = np.zeros(COLS16[layer], np.float16)
    el = np.array([NEG_BIG], np.float32).view(np.float16)
    z[DIN[layer]:DIN[layer] + 2] = el
    return z[None, :]


def _build_bass(S):
    import concourse.bacc as bacc
    import concourse.tile as tile
    import concourse.mybir as mybir

    f32 = mybir.dt.float32
    f16 = mybir.dt.float16
    i16 = mybir.dt.int16
    Alu = mybir.AluOpType
    Act = mybir.ActivationFunctionType

    NIDX = int(128 * S.sum())
    SKIP = set(os.environ.get("K_SKIP", "").split(","))
    nc = bacc.Bacc("TRN2", debug=False, num_devices=NCORES)

    # ---------------- I/O tensors ----------------
    feat_slab = nc.dram_tensor("feat_slab", [RPC, 64], f32, kind="ExternalInput")
    Wt, bt, walr, warr, zrt = [], [], [], [], []
    for l in range(4):
        nch = max(1, DIN[l] // 128)
        kdim = min(128, DIN[l])
        Wt.append(nc.dram_tensor(f"W{l}", [kdim, nch * DOUT[l]], f16, kind="ExternalInput"))
        bt.append(nc.dram_tensor(f"b{l}", [1, DOUT[l]], f16, kind="ExternalInput"))
        walr.append(nc.dram_tensor(f"walr{l}", [128, DIN[l]], f32, kind="ExternalInput"))
        warr.append(nc.dram_tensor(f"warr{l}", [128, DIN[l]], f32, kind="ExternalInput"))
        zrt.append(nc.dram_tensor(f"zr{l}", [1, COLS16[l]], f16, kind="ExternalInput"))
    relWt = nc.dram_tensor("relWp", [128, 8 * 64], f32, kind="ExternalInput")
    relBt = nc.dram_tensor("relB", [1, 64], f32, kind="ExternalInput")
    gidxt = nc.dram_tensor("gidx", [128, NIDX // 16], i16, kind="ExternalInput")
    identt = nc.dram_tensor("ident", [128, 128], f16, kind="ExternalInput")
    outt = nc.dram_tensor("out", [1, 64], f32, kind="ExternalOutput")

    # internal DRAM: per-layer slab (AG input) and full x (AG output)
    slab_t, xfull_t = [], []
    for l in range(4):
        slab_t.append(nc.dram_tensor(f"slab{l}", [RPC, COLS16[l]], f16, kind="Internal"))
        xfull_t.append(nc.dram_tensor(f"xfull{l}", [RPC * NCORES, COLS16[l]], f16,
                                      kind="Internal", addr_space="Shared"))

    RG = [list(range(NCORES))]
    SMAX = int(S.max())

    with tile.TileContext(nc, num_cores=NCORES) as tc:
        with (
            tc.tile_pool(name="const", bufs=1) as constp,
            tc.tile_pool(name="wpool", bufs=1) as wpool,
            tc.tile_pool(name="gpool", bufs=3) as gpool,
            tc.tile_pool(name="work", bufs=3) as work,
            tc.tile_pool(name="small", bufs=4) as small,
            tc.tile_pool(name="scrp", bufs=2) as scrp,
            tc.tile_pool(name="psum", bufs=1, space="PSUM") as psum,
            tc.tile_pool(name="psum2", bufs=2, space="PSUM") as psum2,
            tc.tile_pool(name="psuma", bufs=1, space="PSUM") as psuma,
        ):
            # persistent constants
            gidx_sb = constp.tile([128, NIDX // 16], i16)
            nc.sync.dma_start(gidx_sb[:, :], gidxt[:, :])
            ident_sb = constp.tile([128, 128], f16)
            nc.sync.dma_start(ident_sb[:, :], identt[:, :])
            ones_row = constp.tile([1, 128], f16)
            nc.vector.memset(ones_row[:, :], 1.0)
            ones_col = constp.tile([128, 1], f16)
            nc.vector.memset(ones_col[:, :], 1.0)
            er_s = [constp.tile([128, NW], f32, name=f"er_s{l}") for l in range(4)]
            # pool-engine registers holding 128*S[w] for dma_gather num_idxs
            nidx_sv = {}
            for sw in sorted(set(int(x) for x in S)):
                reg = nc.alloc_register(mybir.EngineType.Pool, f"nidx{sw}")
                nc.gpsimd.reg_mov(reg, 128 * sw)
                nidx_sv[sw] = nc.snap(reg, donate=True)
            zr_sb = []
            for l in range(4):
                z = constp.tile([1, COLS16[l]], f16, name=f"zr_sb{l}")
                nc.sync.dma_start(z[:, :], zrt[l][:, :])
                zr_sb.append(z)

            # ---------------- prep: build layer-0 input slab ----------------
            wal_sb = constp.tile([128, DIN[0]], f32, name="wal0")
            war_sb = constp.tile([128, DIN[0]], f32, name="war0")
            nc.sync.dma_start(wal_sb[:, :], walr[0][:, :])
            nc.sync.dma_start(war_sb[:, :], warr[0][:, :])
            for w in range(NW):
                ft = work.tile([128, 64], f32, tag="ft")
                nc.sync.dma_start(ft[:, :], feat_slab[w * 128:(w + 1) * 128, :])
                aug = work.tile([128, COLS16[0]], f16, tag="aug0")
                scr = work.tile([128, 64], f32, tag="scr0")
                elc = small.tile([128, 1], f32, tag="elc")
                nc.vector.tensor_tensor(out=scr[:, :], in0=ft[:, :],
                                        in1=wal_sb[:, :], op=Alu.mult)
                nc.vector.tensor_reduce(out=elc[:, :], in_=scr[:, :],
                                        op=Alu.add, axis=mybir.AxisListType.X)
                nc.vector.tensor_tensor(out=scr[:, :], in0=ft[:, :],
                                        in1=war_sb[:, :], op=Alu.mult)
                nc.vector.tensor_reduce(out=er_s[0][:, w:w + 1], in_=scr[:, :],
                                        op=Alu.add, axis=mybir.AxisListType.X)
                nc.vector.tensor_copy(aug[:, 0:64], ft[:, :])
                aug32 = aug.bitcast(f32)
                nc.vector.tensor_copy(aug32[:, EL32[0]:EL32[0] + 1], elc[:, :])
                if w == 11:
                    nc.sync.dma_start(aug[PAD_LO - 11 * 128:PAD_LO - 11 * 128 + 1, :],
                                      zr_sb[0][:, :])
                nc.sync.dma_start(slab_t[0][w * 128:(w + 1) * 128, :], aug[:, :])
                if w == 9 and "coll" not in SKIP:
                    nc.gpsimd.collective_compute(
                        "AllGather", Alu.bypass, replica_groups=RG,
                        ins=[slab_t[0][0:HALF, :]],
                        outs=[xfull_t[0][0:NCORES * HALF, :]])
            if "coll" not in SKIP:
                nc.gpsimd.collective_compute(
                    "AllGather", Alu.bypass, replica_groups=RG,
                    ins=[slab_t[0][HALF:RPC, :]],
                    outs=[xfull_t[0][NCORES * HALF:, :]])

            # ---------------- layers ----------------
            pool_row = constp.tile([1, 1024], f32)
            pps = psuma.tile([1, 1024], f32, name="pps")
            for l in range(4):
                din, dout = DIN[l], DOUT[l]
                cols = COLS16[l]
                nch = max(1, din // 128)
                kdim = min(128, din)
                W_sb = wpool.tile([kdim, nch * dout], f16, tag="W")
                nc.sync.dma_start(W_sb[:, :], Wt[l][:, :])
                b_sb = wpool.tile([1, dout], f16, tag="b")
                nc.sync.dma_start(b_sb[:, :], bt[l][:, :])
                if l < 3:
                    waln = wpool.tile([128, DOUT[l]], f32, tag="waln")
                    warn = wpool.tile([128, DOUT[l]], f32, tag="warn")
                    nc.sync.dma_start(waln[:, :], walr[l + 1][:, :])
                    nc.sync.dma_start(warn[:, :], warr[l + 1][:, :])

                base = 0
                for w in range(NW):
                    sw = int(S[w])
                    G = gpool.tile([128, SMAX, cols], f16, tag="G")
                    if "gather" not in SKIP:
                      nc.gpsimd.dma_gather(
                        G[:, 0:sw, :], xfull_t[l][:, :],
                        gidx_sb[:, base // 16:base // 16 + 8 * sw],
                        num_idxs=128 * sw, num_idxs_reg=nidx_sv[sw], elem_size=cols,
                        single_packet=False)
                    base += 128 * sw

                    G32 = G.bitcast(f32)
                    doatt = "att" not in SKIP
                    el_g = G32[:, 0:sw, EL32[l]:EL32[l] + 1]
                    # e = leaky_relu(el + er)
                    t0 = work.tile([128, SMAX, 1], f32, tag="t0")
                    doatt and None; doatt and nc.vector.tensor_scalar_add(t0[:, 0:sw, :], el_g,
                                                er_s[l][:, w:w + 1])
                    t1 = work.tile([128, SMAX, 1], f32, tag="t1")
                    doatt and None; doatt and nc.vector.tensor_scalar_mul(t1[:, 0:sw, :], t0[:, 0:sw, :],
                                                NEG_SLOPE)
                    ee = work.tile([128, SMAX, 1], f32, tag="ee")
                    doatt and None; doatt and nc.vector.tensor_tensor(out=ee[:, 0:sw, :], in0=t0[:, 0:sw, :],
                                            in1=t1[:, 0:sw, :], op=Alu.max)
                    # m = -max(e); ex = exp(e - max); s = sum(ex)
                    mneg = small.tile([128, 1], f32, tag="mneg")
                    doatt and None; doatt and nc.vector.tensor_reduce(out=mneg[:, :], in_=ee[:, 0:sw, :],
                                            op=Alu.max, axis=mybir.AxisListType.XY,
                                            negate=True)
                    ex = work.tile([128, SMAX, 1], f32, tag="ex")
                    ssum = small.tile([128, 1], f32, tag="ssum")
                    doatt and None; doatt and nc.scalar.activation(ex[:, 0:sw, :], ee[:, 0:sw, :], Act.Exp,
                                         bias=mneg[:, :], scale=1.0,
                                         accum_out=ssum[:, :])
                    rs = small.tile([128, 1], f32, tag="rs")
                    doatt and None; doatt and nc.vector.reciprocal(rs[:, :], ssum[:, :])
                    # alpha = ex * rs, one [128, sw] op; then scale each slot
                    # in place (2/3 of slots on DVE 4x mode, 1/3 on ScalarE)
                    if "scale" not in SKIP:
                        alp = work.tile([128, SMAX, 1], f32, tag="alp")
                        nc.vector.tensor_scalar_mul(alp[:, 0:sw, :],
                                                    ex[:, 0:sw, :], rs[:, :])
                        ae = int(os.environ.get("K_ACT_EVERY", "5"))
                        for s in range(sw):
                            if ae and s % ae == ae - 1:
                                nc.scalar.activation(
                                    G[:, s:s + 1, 0:din], G[:, s:s + 1, 0:din],
                                    Act.Copy, scale=alp[:, s:s + 1, 0])
                            else:
                                nc.vector.tensor_scalar_mul(
                                    G[:, s:s + 1, 0:din], G[:, s:s + 1, 0:din],
                                    alp[:, s:s + 1, 0])
                    # agg[v, d] = sum_s G[v, s, d] via pairwise fp16 tree
                    # (tensor_tensor hits DVE 2x mode on fp16; strided
                    # tensor_reduce would be stuck at 1x)
                    agg = work.tile([128, din], f16, tag="agg")
                    if "reduce" not in SKIP:
                        # DVE-only pairwise tree (GPSIMD split tested WORSE:
                        # it delays next-window gather descriptor generation)
                        cnt = sw
                        while cnt > 2:
                            h = cnt // 2
                            nc.vector.tensor_tensor(
                                out=G[:, 0:h, 0:din], in0=G[:, 0:h, 0:din],
                                in1=G[:, cnt - h:cnt, 0:din], op=Alu.add)
                            cnt -= h
                        nc.vector.tensor_tensor(
                            out=agg[:, :], in0=G[:, 0:1, 0:din].rearrange("p s d -> p (s d)"),
                            in1=G[:, 1:2, 0:din].rearrange("p s d -> p (s d)"),
                            op=Alu.add)
                    # transpose agg -> aggT chunks [din, 128v]
                    aggT = work.tile([kdim, nch * 128], f16, tag="aggT")
                    for ci in range(nch):
                        dw = min(128, din - ci * 128)
                        tp = psum.tile([kdim, 128], f16, tag="tp")
                        nc.tensor.transpose(tp[0:dw, :],
                                            agg[:, ci * 128:ci * 128 + dw],
                                            ident_sb[:, :])
                        nc.scalar.copy(aggT[0:dw, ci * 128:(ci + 1) * 128],
                                       tp[0:dw, :])
                    # slab matmul: out[v, n] = sum_d aggT[d, v] * W[d, n] (+ b)
                    ps = psum2.tile([128, dout], f32, tag="ps")
                    nhalf = (dout + 511) // 512
                    for nh in range(nhalf):
                        n0, n1 = nh * 512, min(dout, (nh + 1) * 512)
                        for ci in range(nch):
                            dw = min(128, din - ci * 128)
                            nc.tensor.matmul(
                                ps[:, n0:n1],
                                lhsT=aggT[0:dw, ci * 128:(ci + 1) * 128],
                                rhs=W_sb[0:dw, ci * dout + n0:ci * dout + n1],
                                start=(ci == 0), stop=(ci == nch - 1))
                        nc.tensor.matmul(ps[:, n0:n1], lhsT=ones_row[:, :],
                                         rhs=b_sb[:, n0:n1], start=False, stop=True,
                                         skip_group_check=True)
                    if l < 3:
                        aug = work.tile([128, COLS16[l + 1]], f16, tag="augL")
                        nc.scalar.activation(aug[:, 0:dout], ps[:, :], Act.Tanh)
                        scr = scrp.tile([128, dout], f32, tag="scrL")
                        # el/er for the next layer on GPSIMD (frees DVE)
                        elc = small.tile([128, 1], f32, tag="elcL")
                        eng_el = nc.gpsimd if os.environ.get("K_ELER", "g") == "g" else nc.vector
                        eng_el.tensor_tensor(out=scr[:, :], in0=aug[:, 0:dout],
                                                in1=waln[:, :], op=Alu.mult)
                        nc.vector.tensor_reduce(out=elc[:, :], in_=scr[:, :],
                                                op=Alu.add,
                                                axis=mybir.AxisListType.X)
                        scr2 = scrp.tile([128, dout], f32, tag="scr2")
                        eng_el.tensor_tensor(out=scr2[:, :], in0=aug[:, 0:dout],
                                                in1=warn[:, :], op=Alu.mult)
                        nc.vector.tensor_reduce(out=er_s[l + 1][:, w:w + 1],
                                                in_=scr2[:, :], op=Alu.add,
                                                axis=mybir.AxisListType.X)
                        aug32 = aug.bitcast(f32)
                        nc.vector.tensor_copy(aug32[:, EL32[l + 1]:EL32[l + 1] + 1],
                                              elc[:, :])
                        if w == 11:
                            nc.sync.dma_start(
                                aug[PAD_LO - 11 * 128:PAD_LO - 11 * 128 + 1, :],
                                zr_sb[l + 1][:, :])
                        nc.sync.dma_start(slab_t[l + 1][w * 128:(w + 1) * 128, :],
                                          aug[:, :])
                        if w == 9 and "coll" not in SKIP:
                            nc.gpsimd.collective_compute(
                                "AllGather", Alu.bypass, replica_groups=RG,
                                ins=[slab_t[l + 1][0:HALF, :]],
                                outs=[xfull_t[l + 1][0:NCORES * HALF, :]])
                    else:
                        x4 = work.tile([128, 1024], f16, tag="x4")
                        nc.scalar.activation(x4[:, :], ps[:, :], Act.Tanh)
                        if w >= 12:
                            # pooling: colsum across the 1024 pooling rows
                            for nh in range(2):
                                n0, n1 = nh * 512, (nh + 1) * 512
                                nc.tensor.matmul(
                                    pps[:, n0:n1], lhsT=ones_col[:, :],
                                    rhs=x4[:, n0:n1], start=(w == 12),
                                    stop=(w == 19), skip_group_check=True)
                            if w == 19:
                                nc.vector.tensor_copy(pool_row[:, :], pps[:, :])
                if l < 3 and "coll" not in SKIP:
                    nc.gpsimd.collective_compute(
                        "AllGather", Alu.bypass, replica_groups=RG,
                        ins=[slab_t[l + 1][HALF:RPC, :]],
                        outs=[xfull_t[l + 1][NCORES * HALF:, :]])

            # ---------------- head: logits = pool @ relWp + relB ----------------
            relW_sb = constp.tile([128, 8 * 64], f32)
            nc.sync.dma_start(relW_sb[:, :], relWt[:, :])
            relB_sb = constp.tile([1, 64], f32)
            nc.sync.dma_start(relB_sb[:, :], relBt[:, :])
            one1 = constp.tile([1, 1], f32)
            nc.vector.memset(one1[:, :], 1.0)
            poolT = constp.tile([128, 8], f32)
            # [1, 1024] -> [128, 8] via DRAM bounce: poolT[p, c] = pool[c*128 + p]
            pool_dram = nc.dram_tensor("pool_dram", [1, 1024], f32, kind="Internal")
            nc.sync.dma_start(pool_dram[:, :], pool_row[:, :])
            pdv = pool_dram[:, :].rearrange("o (c p) -> (o p) c", p=128)
            nc.sync.dma_start(poolT[:, :], pdv)
            hps = psuma.tile([1, 64], f32, name="hps")
            for j in range(8):
                nc.tensor.matmul(hps[:, :], lhsT=poolT[:, j:j + 1],
                                 rhs=relW_sb[:, j * 64:(j + 1) * 64],
                                 start=(j == 0), stop=False)
            nc.tensor.matmul(hps[:, :], lhsT=one1[:, :], rhs=relB_sb[:, :],
                             start=False, stop=True, skip_group_check=True)
            out_sb = constp.tile([1, 64], f32)
            nc.vector.tensor_copy(out_sb[:, :], hps[:, :])
            nc.sync.dma_start(outt[:, :], out_sb[:, :])

    nc.compile()
    return nc


def kernel(feat, W1, al1, ar1, b1, W2, al2, ar2, b2, W3, al3, ar3, b3,
           W4, al4, ar4, b4, relW, relB, src, dst, rel, order, **kw):
    feat = np.asarray(feat, np.float32)
    Ws = [np.asarray(W1, np.float32), np.asarray(W2, np.float32),
          np.asarray(W3, np.float32), np.asarray(W4, np.float32)]
    als = [np.asarray(al1, np.float32), np.asarray(al2, np.float32),
           np.asarray(al3, np.float32), np.asarray(al4, np.float32)]
    ars = [np.asarray(ar1, np.float32), np.asarray(ar2, np.float32),
           np.asarray(ar3, np.float32), np.asarray(ar4, np.float32)]
    bs = [np.asarray(b1, np.float32), np.asarray(b2, np.float32),
          np.asarray(b3, np.float32), np.asarray(b4, np.float32)]
    relW = np.asarray(relW, np.float32)
    relB = np.asarray(relB, np.float32)
    src = np.asarray(src, np.int32)
    dst = np.asarray(dst, np.int32)
    rel = np.asarray(rel)

    pos2node, node2row, S, gidx16 = _prep_graph(src, dst)
    nc = _build_bass(S)

    # per-core host inputs
    in_maps = []
    ident = np.eye(128, dtype=np.float16)
    for k in range(NCORES):
        fs = np.zeros((RPC, 64), np.float32)
        m = pos2node[k] >= 0
        fs[np.nonzero(m)[0]] = feat[pos2node[k][m]]
        im = {"feat_slab": fs, "gidx": gidx16[k], "ident": ident,
              "relWp": np.ascontiguousarray(
                  (relW / 1024.0).reshape(8, 128, 64).transpose(1, 0, 2)
              ).reshape(128, 8 * 64),
              "relB": relB[None, :]}
        for l in range(4):
            nch = max(1, DIN[l] // 128)
            kdim = min(128, DIN[l])
            Wl = Ws[l].reshape(nch, kdim, DOUT[l]).transpose(1, 0, 2)
            im[f"W{l}"] = np.ascontiguousarray(Wl).reshape(kdim, nch * DOUT[l]).astype(np.float16)
            im[f"b{l}"] = bs[l][None, :].astype(np.float16)
            wal = Ws[l] @ als[l]
            war = Ws[l] @ ars[l]
            im[f"walr{l}"] = np.tile(wal[None, :], (128, 1)).astype(np.float32)
            im[f"warr{l}"] = np.tile(war[None, :], (128, 1)).astype(np.float32)
            im[f"zr{l}"] = _zrow(l)
        in_maps.append(im)

    from concourse.bass_utils import run_bass_kernel_spmd

    trace = os.environ.get("KERNEL_TRACE", "0") == "1"
    try:
        res = run_bass_kernel_spmd(nc, in_maps, core_ids=list(range(NCORES)),
                                   trace=trace)
    except ModuleNotFoundError:
        res = run_bass_kernel_spmd(nc, in_maps, core_ids=list(range(NCORES)))
    if res.exec_time_ns is not None:
        print(f"HW exec time: {res.exec_time_ns} ns")
        global LAST_EXEC_NS
        LAST_EXEC_NS = res.exec_time_ns
    nbench = int(os.environ.get("KERNEL_BENCH", "0"))
    if nbench:
        import time as _time
        times = []
        for _ in range(nbench):
            t0 = _time.time()
            run_bass_kernel_spmd(nc, in_maps, core_ids=list(range(NCORES)))
            times.append(_time.time() - t0)
        print(f"bench wall times (s): {[round(t, 3) for t in times]}")
        global LAST_BENCH_S
        LAST_BENCH_S = min(times)
    logits = res.results[0]["out"][0]

    nz = np.flatnonzero(np.asarray(rel))
    nz = np.concatenate([nz, np.zeros(max(0, rel.shape[0] - nz.size), np.int64)])
    return logits[nz].astype(np.float32)


LAST_EXEC_NS = None
LAST_BENCH_S = None

